# revision 1
# baseline (speedup 1.0000x reference)
"""Multi-head attention (B=2, T=2048, d_model=1024, H=16, hd=64) on 8 Trainium2
NeuronCores.

Sharding: the 32 (batch, head) attention units are split as 4 consecutive heads
of one batch per core (core c -> batch c//4, heads 4*(c%4) .. 4*(c%4)+3). Each
core computes its own QKV projection slice, causal attention for its heads, and
a partial out-projection (its 256 rows of W_out). The host sums the 4 partials
per batch and adds b_out.

Device-side layout (everything flows transposed so no on-chip transposes are
needed until the attention output):
  qT/kT [hd, T]  <- lhsT=W slice, rhs=xT
  v     [T, hd]  (+ ones column for the row-sum trick)
  sT    [k, q]   <- lhsT=kT chunk, rhs=qT          (psum, fp32)
  E     [k, q]   <- exp(sT * 1/sqrt(hd)) on ScalarE (bf16)
  pv    [q, hd+1]<- lhsT=E chunk, rhs=[v|1]        (col hd = row sum)
  a     [q, hd]  = pv[:, :hd] * (1/pv[:, hd])      (per-partition scalar)
  aT    [hd, T]  via DRAM round-trip DMA transpose
  out  += aT.T @ W_out slice                        (partial, fp32)
"""

import math
import os
from contextlib import ExitStack
from dataclasses import dataclass

import numpy as np
import ml_dtypes

import concourse.bass as bass
import concourse.tile as tile
from concourse import bacc, mybir
from concourse import bass_utils

AF = mybir.ActivationFunctionType
ALU = mybir.AluOpType
DT = mybir.dt

N_CORES = 8
NEG = -1e9


@dataclass(frozen=True)
class Cfg:
    T: int = 2048        # sequence length
    DM: int = 1024       # d_model
    HD: int = 64         # head dim
    NH: int = 4          # heads per core
    mode: str = "causal"  # "causal" | "full" | "bias"
    mm: str = "bf16"     # matmul operand dtype: "bf16" | "f32r" | "f32"

    @property
    def NHD(self):
        return self.NH * self.HD          # qkv slice width per core

    @property
    def KC(self):
        return self.DM // 128             # contraction chunks for projections

    @property
    def MC(self):
        return self.NHD // 128            # qT/kT partition chunks

    @property
    def TC(self):
        return self.T // 128              # t chunks

    @property
    def QW(self):
        return min(512, self.T)           # q group width

    @property
    def QG(self):
        return self.T // self.QW

    @property
    def QT(self):
        return self.QW // 128             # q tiles per group

    @property
    def EB(self):
        return self.DM // 512             # out-proj free blocks

    @property
    def mmdt(self):
        return {"bf16": DT.bfloat16, "f32r": DT.float32r, "f32": DT.float32}[self.mm]

    @property
    def npmm(self):
        return ml_dtypes.bfloat16 if self.mm == "bf16" else np.float32


def build_program(cfg: Cfg):
    """Build + compile the SPMD single-core program. Returns (nc, input_names)."""
    c = cfg
    assert c.DM % 128 == 0 and c.NHD % 128 == 0 and c.T % 512 == 0
    nc = bacc.Bacc("TRN2", target_bir_lowering=False, debug=False,
                   num_devices=N_CORES)
    f32 = DT.float32
    bf16 = DT.bfloat16
    mmdt = c.mmdt

    xT = nc.dram_tensor("xT", [c.DM, c.T], mmdt, kind="ExternalInput").ap()
    wq = nc.dram_tensor("wq", [c.DM, c.NHD], mmdt, kind="ExternalInput").ap()
    wk = nc.dram_tensor("wk", [c.DM, c.NHD], mmdt, kind="ExternalInput").ap()
    wv = nc.dram_tensor("wv", [c.DM, c.NHD], mmdt, kind="ExternalInput").ap()
    bq = nc.dram_tensor("bq", [128, c.MC], f32, kind="ExternalInput").ap()
    bk = nc.dram_tensor("bk", [128, c.MC], f32, kind="ExternalInput").ap()
    bvb = nc.dram_tensor("bvb", [128, c.NHD], f32, kind="ExternalInput").ap()
    wo = nc.dram_tensor("wo", [c.NHD, c.DM], mmdt, kind="ExternalInput").ap()
    maskb = None
    if c.mode == "bias":
        # additive bias, transposed: maskb[k, q]
        maskb = nc.dram_tensor("maskb", [c.T, c.T], f32, kind="ExternalInput").ap()
    out = nc.dram_tensor("out", [c.T, c.DM], f32, kind="ExternalOutput").ap()

    with tile.TileContext(nc) as tc, ExitStack() as ctx:
        _body(ctx, tc, c, xT, wq, wk, wv, bq, bk, bvb, wo, maskb, out)
    nc.compile()
    names = ["xT", "wq", "wk", "wv", "bq", "bk", "bvb", "wo"]
    if c.mode == "bias":
        names.append("maskb")
    return nc, names


def _body(ctx, tc, c: Cfg, xT, wq, wk, wv, bq, bk, bvb, wo, maskb, out):
    nc = tc.nc
    f32 = DT.float32
    bf16 = DT.bfloat16
    mmdt = c.mmdt
    causal = c.mode == "causal"
    scale = 1.0 / math.sqrt(c.HD)

    const = ctx.enter_context(tc.tile_pool(name="const", bufs=1))
    big = ctx.enter_context(tc.tile_pool(name="big", bufs=1))
    epool = ctx.enter_context(tc.tile_pool(name="E", bufs=c.TC))
    rpool = ctx.enter_context(tc.tile_pool(name="r", bufs=8))
    # PSUM: 3 x [128,1024] (6 banks) + 2 x [128,65] (2 banks) = 8 banks
    ps_mm = ctx.enter_context(tc.tile_pool(name="psmm", bufs=3, space="PSUM"))
    ps_pv = ctx.enter_context(tc.tile_pool(name="pspv", bufs=2, space="PSUM"))
    dramp = ctx.enter_context(tc.tile_pool(name="dram", bufs=1, space="DRAM"))
    bias_pool = None
    if c.mode == "bias":
        bias_pool = ctx.enter_context(tc.tile_pool(name="maskb", bufs=4))

    # ---- load inputs to SBUF ----
    bq_sb = const.tile([128, c.MC], f32, tag="bq")
    nc.sync.dma_start(out=bq_sb[:], in_=bq)
    bk_sb = const.tile([128, c.MC], f32, tag="bk")
    nc.sync.dma_start(out=bk_sb[:], in_=bk)
    bvb_sb = const.tile([128, c.NHD], f32, tag="bvb")
    nc.sync.dma_start(out=bvb_sb[:], in_=bvb)

    # consolidated input DMAs (one 3D-AP transfer each) — per-dma descriptor
    # generation on the sync sequencer is ~0.6us, so fewer, bigger DMAs
    # split along t so the first QKV block (which contracts over ALL chunks)
    # can start after the first half arrives
    xT_sb = big.tile([128, c.KC, c.T], mmdt, tag="xT")
    xTd = xT.rearrange("(c p) t -> p c t", p=128)
    TH = max(512, c.T // 2)
    for h in range(c.T // TH):
        nc.sync.dma_start(out=xT_sb[:, :, h * TH:(h + 1) * TH],
                          in_=xTd[:, :, h * TH:(h + 1) * TH])

    w_sbs = []
    for nm, w in (("wq", wq), ("wk", wk), ("wv", wv)):
        w_sb = big.tile([128, c.KC, c.NHD], mmdt, tag=nm)
        nc.sync.dma_start(out=w_sb[:],
                          in_=w.rearrange("(c p) n -> p c n", p=128))
        w_sbs.append(w_sb)
    wq_sb, wk_sb, wv_sb = w_sbs

    wo_sb = big.tile([128, c.MC, c.DM], mmdt, tag="wo")
    nc.sync.dma_start(out=wo_sb[:],
                      in_=wo.rearrange("(c p) n -> p c n", p=128))

    # causal mask block for diagonal tiles: tri[k, j] = 0 if j >= k else NEG
    tri = const.tile([128, 128], f32, tag="tri")
    nc.gpsimd.memset(tri[:], 0.0)
    nc.gpsimd.affine_select(
        out=tri[:], in_=tri[:],
        compare_op=ALU.is_ge, fill=NEG,
        base=0, channel_multiplier=-1, pattern=[[1, 128]],
    )

    # ---- QKV projections ----
    # psum tiles are [128, 1024] (2 banks); two 512-wide matmul groups per
    # tile, one wide DVE biased copy out.
    # qT is stored zero-padded per head ([128, NH, T], head h in partitions
    # (h%2)*64..+64, zeros elsewhere) so score matmuls can run with full
    # K=128 contraction: the other head's kT rows hit zeros. Full-K matmuls
    # keep the PE activity monitor busy -> 2.4 GHz instead of 1.2.
    qT_z = big.tile([128, c.NH, c.T], mmdt, tag="qT")
    nc.vector.memset(qT_z[:], 0.0)
    kT_sb = big.tile([128, c.MC, c.T], mmdt, tag="kT")
    HD1 = c.HD + 1
    v_sb = big.tile([128, c.TC, c.NH, HD1], bf16, tag="v")
    nc.vector.memset(v_sb[:, :, :, c.HD:HD1], 1.0)
    W2 = min(1024, c.T)
    VG = min(c.TC, max(1, 1024 // c.NHD))    # t-chunks per v psum tile

    def emit_qk_tile(m, w_sb, b_sb, which, n):
        ps = ps_mm.tile([128, 1024], f32, tag="mm")
        for d in range(W2 // 512):
            for k in range(c.KC):
                nc.tensor.matmul(
                    ps[:, d * 512:(d + 1) * 512],
                    lhsT=w_sb[:, k, m * 128:(m + 1) * 128],
                    rhs=xT_sb[:, k, n * W2 + d * 512:n * W2 + (d + 1) * 512],
                    start=(k == 0), stop=(k == c.KC - 1),
                )
        sl = slice(n * W2, (n + 1) * W2)
        if which == "k":
            nc.vector.tensor_scalar_add(
                kT_sb[:, m, sl], ps[:, 0:W2], b_sb[:, m:m + 1],
            )
        else:
            nc.vector.tensor_scalar_add(
                qT_z[0:64, 2 * m, sl], ps[0:64, 0:W2], b_sb[0:64, m:m + 1],
            )
            nc.vector.tensor_scalar_add(
                qT_z[64:128, 2 * m + 1, sl], ps[64:128, 0:W2],
                b_sb[64:128, m:m + 1],
            )

    def emit_qk(m):
        for w_sb, b_sb, which in ((wq_sb, bq_sb, "q"), (wk_sb, bk_sb, "k")):
            for n in range(c.T // W2):
                emit_qk_tile(m, w_sb, b_sb, which, n)

    def emit_v_tile(tg):
        # v in normal layout, augmented with a ones column per head;
        # VG t-chunks share one psum tile.
        ps = ps_mm.tile([128, 1024], f32, tag="mm")
        for d in range(VG):
            t = tg * VG + d
            for k in range(c.KC):
                nc.tensor.matmul(
                    ps[:, d * c.NHD:(d + 1) * c.NHD],
                    lhsT=xT_sb[:, k, t * 128:(t + 1) * 128],
                    rhs=wv_sb[:, k, :],
                    start=(k == 0), stop=(k == c.KC - 1),
                )
        for d in range(VG):
            t = tg * VG + d
            nc.vector.tensor_tensor(
                out=v_sb[:, t, :, 0:c.HD],
                in0=ps[:, d * c.NHD:(d + 1) * c.NHD].rearrange(
                    "p (h d) -> p h d", d=c.HD),
                in1=bvb_sb.rearrange("p (h d) -> p h d", d=c.HD),
                op=ALU.add,
            )

    # (emission of qk/v/attention is interleaved below: head-pair hp's
    # attention is emitted before chunk hp+1's q/k so the psum-slot FIFO
    # doesn't serialize attention behind the whole projection phase)

    # ---- attention (head-pair outer, q-group inner) ----
    # kc chunks are paired into [128, 1024] psum tiles so one exp covers
    # 1024 columns. Scores run with full K=128 contraction against the
    # natural two-head kT chunk (zero-padded qT kills the other head's
    # contribution), which keeps the PE activity monitor at 2.4 GHz.
    a_sb = big.tile([128, c.TC, c.NH, c.HD], bf16, tag="a")
    a_dram = dramp.tile([c.T, c.NHD], bf16, tag="adram")
    aT_sb = big.tile([128, c.MC, c.T], bf16, tag="aT")
    ostage = ctx.enter_context(tc.tile_pool(name="ostage", bufs=4))

    def attn_hp(hp, fillers=()):
        fillers = list(fillers)
        per_g = -(-len(fillers) // c.QG) if fillers else 0
        for g in range(c.QG):
            for _ in range(per_g):
                if fillers:
                    fillers.pop(0)()
            kmax = (g + 1) * c.QT if causal else c.TC
            assert kmax % 2 == 0
            etiles = {}                      # (hl, kp) -> [128, 1024] E tile
            for kp in range(kmax // 2):
                for hl in range(2):
                    h = 2 * hp + hl
                    ps = ps_mm.tile([128, 1024], f32, tag="mm")
                    for d in range(2):
                        kc = 2 * kp + d
                        nc.tensor.matmul(
                            ps[:, d * 512:d * 512 + c.QW],
                            lhsT=kT_sb[:, hp, kc * 128:(kc + 1) * 128],
                            rhs=qT_z[:, h, g * c.QW:(g + 1) * c.QW],
                            start=True, stop=True,
                        )
                        if causal:
                            off = (kc - g * c.QT) * 128
                            if off >= 0:
                                nc.vector.tensor_tensor(
                                    out=ps[:, d * 512 + off:d * 512 + off + 128],
                                    in0=ps[:, d * 512 + off:d * 512 + off + 128],
                                    in1=tri[:], op=ALU.add,
                                )
                        elif c.mode == "bias":
                            mb = bias_pool.tile([128, c.QW], f32, tag="mb")
                            nc.sync.dma_start(
                                out=mb[:],
                                in_=maskb[kc * 128:(kc + 1) * 128,
                                          g * c.QW:(g + 1) * c.QW],
                            )
                            nc.vector.tensor_tensor(
                                out=ps[:, d * 512:d * 512 + c.QW],
                                in0=ps[:, d * 512:d * 512 + c.QW],
                                in1=mb[:], op=ALU.add,
                            )
                    et = epool.tile([128, 1024], bf16, tag="E")
                    nc.scalar.activation(et[:], ps[:], AF.Exp, scale=scale)
                    etiles[(hl, kp)] = et
            for hl in range(2):
                h = 2 * hp + hl
                for j in range(c.QT):
                    qt = g * c.QT + j
                    kn = qt + 1 if causal else c.TC
                    psv = ps_pv.tile([128, HD1], f32, tag="pv")
                    for kc in range(kn):
                        kp, d = divmod(kc, 2)
                        nc.tensor.matmul(
                            psv[:],
                            lhsT=etiles[(hl, kp)][
                                :, d * 512 + j * 128:d * 512 + (j + 1) * 128],
                            rhs=v_sb[:, kc, h, :],
                            start=(kc == 0), stop=(kc == kn - 1),
                        )
                    r = rpool.tile([128, 1], f32, tag="r")
                    nc.vector.reciprocal(r[:], psv[:, c.HD:HD1])
                    nc.vector.tensor_scalar_mul(
                        a_sb[:, qt, h, :], psv[:, 0:c.HD], r[:, 0:1],
                    )
                    # stream a out to DRAM as soon as each t-chunk is done
                    if hp == c.NH // 2 - 1 and hl == 1:
                        nc.sync.dma_start(
                            out=a_dram[qt * 128:(qt + 1) * 128, :],
                            in_=a_sb[:, qt, :, :],
                        )

            # transpose this group's a rows -> aT as soon as they're final
            if hp == c.NH // 2 - 1:
                for ci in range(c.MC):
                    nc.sync.dma_start(
                        out=aT_sb[:, ci, g * c.QW:(g + 1) * c.QW],
                        in_=a_dram[g * c.QW:(g + 1) * c.QW,
                                   ci * 128:(ci + 1) * 128],
                        transpose=True,
                    )

    # Head pair hp needs only q/k chunk hp (+v). Emit pair hp+1's projection
    # tiles as fillers inside pair hp's attention groups so they overlap the
    # ACT-bound exp phase instead of serializing behind it in the psum FIFO.
    emit_qk(0)
    for tg in range(c.TC // VG):
        emit_v_tile(tg)
    for m in range(1, c.NH // 2):
        emit_qk(m)
    for hp in range(c.NH // 2):
        attn_hp(hp)

    # ---- partial out-projection ----
    EW = min(1024, c.DM)
    for t in range(c.TC):
        for ebg in range(c.DM // EW):
            ps = ps_mm.tile([128, 1024], f32, tag="mm")
            for d in range(EW // 512):
                e0 = ebg * EW + d * 512
                for ci in range(c.MC):
                    nc.tensor.matmul(
                        ps[:, d * 512:(d + 1) * 512],
                        lhsT=aT_sb[:, ci, t * 128:(t + 1) * 128],
                        rhs=wo_sb[:, ci, e0:e0 + 512],
                        start=(ci == 0), stop=(ci == c.MC - 1),
                    )
            ot = ostage.tile([128, EW], f32, tag="o")
            if t % 2 == 0:
                nc.vector.tensor_copy(ot[:], ps[:, 0:EW])
            else:
                nc.scalar.copy(ot[:], ps[:, 0:EW])
            nc.sync.dma_start(
                out=out[t * 128:(t + 1) * 128, ebg * EW:(ebg + 1) * EW],
                in_=ot[:],
            )


# ---------------------------------------------------------------------------
# host side
# ---------------------------------------------------------------------------

_CACHE: dict = {}


def _get_program(cfg: Cfg):
    key = cfg
    if key not in _CACHE:
        _CACHE[key] = build_program(cfg)
    return _CACHE[key]


def _mask_mode(mask: np.ndarray, T: int) -> str:
    m = (np.asarray(mask).reshape(T, T) != 0)
    if m.all():
        return "full"
    if np.array_equal(m, np.tril(np.ones((T, T), dtype=bool))):
        return "causal"
    return "bias"


def make_in_maps(cfg: Cfg, x, W_qkv, b_qkv, W_out, mask=None):
    """Slice full inputs into the 8 per-core input dicts."""
    c = cfg
    npmm = c.npmm
    B = x.shape[0]
    n_hg = N_CORES // B                      # head groups per batch
    in_maps = []
    maskb = None
    if c.mode == "bias":
        m = (np.asarray(mask).reshape(c.T, c.T) != 0)
        maskb = np.where(m, np.float32(0), np.float32(NEG)).T.copy()
    for core in range(N_CORES):
        b, hg = divmod(core, n_hg)
        col0 = hg * c.NHD
        xT = np.ascontiguousarray(x[b].T).astype(npmm)
        wq_ = np.ascontiguousarray(W_qkv[:, 0 * c.DM + col0:0 * c.DM + col0 + c.NHD]).astype(npmm)
        wk_ = np.ascontiguousarray(W_qkv[:, 1 * c.DM + col0:1 * c.DM + col0 + c.NHD]).astype(npmm)
        wv_ = np.ascontiguousarray(W_qkv[:, 2 * c.DM + col0:2 * c.DM + col0 + c.NHD]).astype(npmm)
        bq_ = np.ascontiguousarray(
            b_qkv[0 * c.DM + col0:0 * c.DM + col0 + c.NHD].reshape(c.MC, 128).T
        ).astype(np.float32)
        bk_ = np.ascontiguousarray(
            b_qkv[1 * c.DM + col0:1 * c.DM + col0 + c.NHD].reshape(c.MC, 128).T
        ).astype(np.float32)
        bv_ = b_qkv[2 * c.DM + col0:2 * c.DM + col0 + c.NHD].astype(np.float32)
        bvb_ = np.ascontiguousarray(np.broadcast_to(bv_, (128, c.NHD)))
        wo_ = np.ascontiguousarray(W_out[col0:col0 + c.NHD, :]).astype(npmm)
        im = dict(xT=xT, wq=wq_, wk=wk_, wv=wv_, bq=bq_, bk=bk_, bvb=bvb_,
                  wo=wo_)
        if c.mode == "bias":
            im["maskb"] = maskb
        in_maps.append(im)
    return in_maps


def run_sharded(cfg: Cfg, x, W_qkv, b_qkv, W_out, b_out, mask=None, **kw):
    """Run the SPMD program on 8 cores and assemble the full output."""
    nc, _names = _get_program(cfg)
    in_maps = make_in_maps(cfg, x, W_qkv, b_qkv, W_out, mask)
    res = bass_utils.run_bass_kernel_spmd(
        nc, in_maps, core_ids=list(range(N_CORES)), **kw,
    )
    outs = [r["out"] for r in res.results]
    B = x.shape[0]
    n_hg = N_CORES // B
    y = np.stack([
        np.sum(outs[b * n_hg:(b + 1) * n_hg], axis=0) for b in range(B)
    ]) + b_out.astype(np.float32)
    return y.astype(np.float32), res


def kernel(x, W_qkv, b_qkv, W_out, b_out, mask):
    x = np.asarray(x, dtype=np.float32)
    W_qkv = np.asarray(W_qkv, dtype=np.float32)
    b_qkv = np.asarray(b_qkv, dtype=np.float32)
    W_out = np.asarray(W_out, dtype=np.float32)
    b_out = np.asarray(b_out, dtype=np.float32)
    B, T, DM = x.shape
    mode = _mask_mode(mask, T)
    cfg = Cfg(T=T, DM=DM, mode=mode, mm=os.environ.get("MHA_MM_DT", "bf16"))
    y, _ = run_sharded(cfg, x, W_qkv, b_qkv, W_out, b_out, mask)
    return y



# revision 4
# speedup vs baseline: 1.0088x; 1.0088x over previous
"""Multi-head attention (B=2, T=2048, d_model=1024, H=16, hd=64) on 8 Trainium2
NeuronCores.

Sharding: 32 (batch, head) units -> 4 consecutive heads of one batch per core.
Each core: QKV projection slice, causal attention for its heads, partial
out-projection (its 256 rows of W_out). Host sums 4 partials/batch + b_out.

v2 design (vs v1):
  - scores: two concurrent half-array K=64 matmuls (head pair at partition
    bases 0/64) instead of zero-padded K=128 -> ~2x score throughput.
  - scores land in bf16 psum tiles [128, 4*512] (4 k-chunks, 2 banks) so one
    ACT exp instruction covers 4 chunks (amortizes 352-cyc ACT overhead).
  - attn@V transposed: lhsT = v[128, hd+1] stationary (cheap LDWEIGHTS),
    rhs = E streams N=512 -> psum pvT[65, 512] = aT rows + rowsum row.
    No DRAM round-trip transpose, no per-q-tile tiny matmuls.
  - diagonal narrowing: score/pv matmuls skip fully-masked column ranges;
    masked E columns are never read, so no zero fills.
  - softmax normalize in transposed space: DVE reciprocal of rowsum row,
    gpsimd partition_broadcast, DVE multiply -> aT sbuf (bf16).
  - g-outer loop with a deadline-driven filler queue (projection slices,
    out-projection t-chunks) to keep PE busy while ACT (exp) is the
    bottleneck engine.
"""

import math
from contextlib import ExitStack
from dataclasses import dataclass

import numpy as np
import ml_dtypes

import concourse.bass as bass
import concourse.tile as tile
from concourse import bacc, mybir
from concourse import bass_utils

AF = mybir.ActivationFunctionType
ALU = mybir.AluOpType
DT = mybir.dt

N_CORES = 8
NEG = -1e9


@dataclass(frozen=True)
class Cfg:
    T: int = 2048        # sequence length
    DM: int = 1024       # d_model
    HD: int = 64         # head dim
    NH: int = 4          # heads per core
    QW: int = 512        # q group width
    scores_bf16: bool = False  # bf16 psum matmul output unsupported by bass

    @property
    def NHD(self):
        return self.NH * self.HD          # qkv slice width per core

    @property
    def KC(self):
        return self.DM // 128             # contraction chunks for projections

    @property
    def MC(self):
        return self.NH // 2               # head pairs per core

    @property
    def TC(self):
        return self.T // 128              # t chunks

    @property
    def QG(self):
        return self.T // self.QW          # q groups

    @property
    def QT(self):
        return self.QW // 128             # new k chunks per q group


def build_program(cfg: Cfg):
    c = cfg
    nc = bacc.Bacc("TRN2", target_bir_lowering=False, debug=False,
                   num_devices=N_CORES)
    f32 = DT.float32
    bf16 = DT.bfloat16

    xT = nc.dram_tensor("xT", [c.DM, c.T], bf16, kind="ExternalInput").ap()
    wq = nc.dram_tensor("wq", [c.DM, c.NHD], bf16, kind="ExternalInput").ap()
    wk = nc.dram_tensor("wk", [c.DM, c.NHD], bf16, kind="ExternalInput").ap()
    wv = nc.dram_tensor("wv", [c.DM, c.NHD], bf16, kind="ExternalInput").ap()
    bq = nc.dram_tensor("bq", [128, c.MC], f32, kind="ExternalInput").ap()
    bk = nc.dram_tensor("bk", [128, c.MC], f32, kind="ExternalInput").ap()
    bvb = nc.dram_tensor("bvb", [128, c.NHD], f32, kind="ExternalInput").ap()
    wo = nc.dram_tensor("wo", [c.NHD, c.DM], bf16, kind="ExternalInput").ap()
    out = nc.dram_tensor("out", [c.T, c.DM], f32, kind="ExternalOutput").ap()

    with tile.TileContext(nc) as tc, ExitStack() as ctx:
        _body(ctx, tc, c, xT, wq, wk, wv, bq, bk, bvb, wo, out)
    nc.compile()
    return nc


def _body(ctx, tc, c: Cfg, xT, wq, wk, wv, bq, bk, bvb, wo, out):
    nc = tc.nc
    f32 = DT.float32
    bf16 = DT.bfloat16
    scale = 1.0 / math.sqrt(c.HD)
    HD1 = c.HD + 1
    QW = c.QW
    QT = c.QT

    const = ctx.enter_context(tc.tile_pool(name="const", bufs=1))
    big = ctx.enter_context(tc.tile_pool(name="big", bufs=1))
    epool = ctx.enter_context(tc.tile_pool(name="E", bufs=8))
    npool = ctx.enter_context(tc.tile_pool(name="norm", bufs=4))
    ostage = ctx.enter_context(tc.tile_pool(name="ostage", bufs=3))
    # PSUM budget (8 banks): pmm 2x2 + pfp 2x1 + ppv 2x1 = 8
    sdt = bf16 if c.scores_bf16 else f32
    SCH = 4 if c.scores_bf16 else 2          # score k-chunks per psum tile
    pmm = ctx.enter_context(tc.tile_pool(name="pmm", bufs=2, space="PSUM"))
    pfp = ctx.enter_context(tc.tile_pool(name="pfp", bufs=2, space="PSUM"))
    ppv = ctx.enter_context(tc.tile_pool(name="ppv", bufs=2, space="PSUM"))

    # ---- input DMAs ----
    bq_sb = const.tile([128, c.MC], f32, tag="bq")
    nc.sync.dma_start(out=bq_sb[:], in_=bq)
    bk_sb = const.tile([128, c.MC], f32, tag="bk")
    nc.sync.dma_start(out=bk_sb[:], in_=bk)
    bvb_sb = const.tile([128, c.NHD], f32, tag="bvb")
    nc.sync.dma_start(out=bvb_sb[:], in_=bvb)

    w_sbs = []
    for nm, w in (("wq", wq), ("wk", wk), ("wv", wv)):
        w_sb = big.tile([128, c.KC, c.NHD], bf16, tag=nm)
        nc.sync.dma_start(out=w_sb[:],
                          in_=w.rearrange("(c p) n -> p c n", p=128))
        w_sbs.append(w_sb)
    wq_sb, wk_sb, wv_sb = w_sbs

    # xT in column halves so first projections can start early
    xT_sb = big.tile([128, c.KC, c.T], bf16, tag="xT")
    xTd = xT.rearrange("(c p) t -> p c t", p=128)
    TH = max(512, c.T // 2)
    for h in range(c.T // TH):
        nc.sync.dma_start(out=xT_sb[:, :, h * TH:(h + 1) * TH],
                          in_=xTd[:, :, h * TH:(h + 1) * TH])

    wo_sb = big.tile([128, c.MC, c.DM], bf16, tag="wo")
    nc.sync.dma_start(out=wo_sb[:],
                      in_=wo.rearrange("(c p) n -> p c n", p=128))

    # causal mask block: tri[k, j] = 0 if j >= k else NEG
    tri = const.tile([128, 128], f32, tag="tri")
    nc.gpsimd.memset(tri[:], 0.0)
    nc.gpsimd.affine_select(
        out=tri[:], in_=tri[:],
        compare_op=ALU.is_ge, fill=NEG,
        base=0, channel_multiplier=-1, pattern=[[1, 128]],
    )

    # exp table-load warmup (overlaps the input DMA phase)
    warm = const.tile([1, 16], f32, tag="warm")
    nc.vector.memset(warm[:], 0.0)
    nc.scalar.activation(warm[:], warm[:], AF.Exp, scale=1.0)

    # persistent activations
    qT_sb = big.tile([128, c.MC, c.T], bf16, tag="qT")   # pair layout
    kT_sb = big.tile([128, c.MC, c.T], bf16, tag="kT")
    v_sb = big.tile([128, c.TC, c.NH, HD1], bf16, tag="v")
    nc.vector.memset(v_sb[:, :, :, c.HD:HD1], 1.0)
    aT_sb = big.tile([128, c.MC, c.T], bf16, tag="aT")

    # ---- projection / out-proj emitters (pfp [128,512] one-bank tiles) ----
    def emit_qk(m, which, n):
        """q or k for pair m, columns [n*512, (n+1)*512)."""
        w_sb, b_sb = (wq_sb, bq_sb) if which == "q" else (wk_sb, bk_sb)
        ps = pfp.tile([128, 512], f32, tag="fp")
        for k in range(c.KC):
            nc.tensor.matmul(
                ps[:],
                lhsT=w_sb[:, k, m * 128:(m + 1) * 128],
                rhs=xT_sb[:, k, n * 512:(n + 1) * 512],
                start=(k == 0), stop=(k == c.KC - 1),
            )
        dst = qT_sb if which == "q" else kT_sb
        nc.vector.tensor_scalar_add(
            dst[:, m, n * 512:(n + 1) * 512], ps[:], b_sb[:, m:m + 1],
        )

    def emit_v(tg):
        """v for t-chunks [tg, tg+2)."""
        ps = pfp.tile([128, 512], f32, tag="fp")
        for d in range(2):
            t = tg + d
            for k in range(c.KC):
                nc.tensor.matmul(
                    ps[:, d * c.NHD:(d + 1) * c.NHD],
                    lhsT=xT_sb[:, k, t * 128:(t + 1) * 128],
                    rhs=wv_sb[:, k, :],
                    start=(k == 0), stop=(k == c.KC - 1),
                )
        for d in range(2):
            t = tg + d
            nc.vector.tensor_tensor(
                out=v_sb[:, t, :, 0:c.HD],
                in0=ps[:, d * c.NHD:(d + 1) * c.NHD].rearrange(
                    "p (h d) -> p h d", d=c.HD),
                in1=bvb_sb.rearrange("p (h d) -> p h d", d=c.HD),
                op=ALU.add,
            )

    def emit_outproj(t, half):
        """partial out-projection for t-chunk t, output cols half*512.
        half==1 also DMAs the completed row block."""
        ps = pfp.tile([128, 512], f32, tag="fp")
        for ci in range(c.MC):
            nc.tensor.matmul(
                ps[:],
                lhsT=aT_sb[:, ci, t * 128:(t + 1) * 128],
                rhs=wo_sb[:, ci, half * 512:(half + 1) * 512],
                start=(ci == 0), stop=(ci == c.MC - 1),
            )
        key = t % 3
        if half == 0:
            ot = ostage.tile([128, 1024], f32, tag="o")
            _ostage[key] = ot
        else:
            ot = _ostage[key]
        nc.vector.tensor_copy(ot[:, half * 512:(half + 1) * 512], ps[:])
        if half == 1:
            nc.sync.dma_start(out=out[t * 128:(t + 1) * 128, :], in_=ot[:])

    _ostage = {}

    # ---- attention ----
    def attn_pair(hp, g, fillers):
        kmax = (g + 1) * QT
        NT = -(-kmax // SCH)
        gq = g * QW
        he, ho = 2 * hp, 2 * hp + 1
        pv_e = ppv.tile([65, QW], f32, tag="pv")
        pv_o = ppv.tile([65, QW], f32, tag="pv")
        etiles = []

        def emit_scores(i):
            kc0 = i * SCH
            nch = min(SCH, kmax - kc0)
            psE = pmm.tile([128, SCH * 512], sdt, tag="mm")
            psO = pmm.tile([128, SCH * 512], sdt, tag="mm")
            for j in range(nch):
                kc = kc0 + j
                off = max(0, (kc - g * QT) * 128)   # fully-masked prefix
                for base, ps in ((0, psE), (64, psO)):
                    nc.tensor.matmul(
                        ps[:, j * 512 + off:(j + 1) * 512],
                        lhsT=kT_sb[base:base + 64, hp,
                                   kc * 128:(kc + 1) * 128],
                        rhs=qT_sb[base:base + 64, hp, gq + off:gq + QW],
                        start=True, stop=True,
                    )
            for j in range(nch):
                kc = kc0 + j
                off = (kc - g * QT) * 128
                if off >= 0:                        # diagonal chunk
                    for ps in (psE, psO):
                        nc.vector.tensor_tensor(
                            out=ps[:, j * 512 + off:j * 512 + off + 128],
                            in0=ps[:, j * 512 + off:j * 512 + off + 128],
                            in1=tri[:], op=ALU.add,
                        )
            ee = epool.tile([128, SCH * 512], bf16, tag="E")
            nc.scalar.activation(ee[:, 0:nch * 512], psE[:, 0:nch * 512],
                                 AF.Exp, scale=scale)
            eo = epool.tile([128, SCH * 512], bf16, tag="E")
            nc.scalar.activation(eo[:, 0:nch * 512], psO[:, 0:nch * 512],
                                 AF.Exp, scale=scale)
            etiles.append((ee, eo, kc0, nch))

        def emit_pv(i):
            ee, eo, kc0, nch = etiles[i]
            for et, h, pv in ((ee, he, pv_e), (eo, ho, pv_o)):
                for j in range(nch):
                    kc = kc0 + j
                    off = max(0, (kc - g * QT) * 128)
                    nc.tensor.matmul(
                        pv[:, off:QW],
                        lhsT=v_sb[:, kc, h, :],
                        rhs=et[:, j * 512 + off:(j + 1) * 512],
                        start=(kc == 0), stop=(kc == kmax - 1),
                    )

        for i in range(NT):
            emit_scores(i)
            while fillers and fillers[0][0] <= (g, hp, i):
                fillers.pop(0)[1]()
            if i >= 1:
                emit_pv(i - 1)
        emit_pv(NT - 1)

        # normalize: aT[d, q] = pvT[d, q] * (1 / pvT[64, q])
        rcs, bcs = [], []
        for pv in (pv_e, pv_o):
            rc = npool.tile([1, QW], f32, tag="rc")
            nc.vector.reciprocal(rc[:], pv[64:65, :])
            rcs.append(rc)
        for rc in rcs:
            bc = npool.tile([64, QW], f32, tag="bc")
            nc.gpsimd.partition_broadcast(bc[:], rc[0:1, :])
            bcs.append(bc)
        for hl, (pv, bc) in enumerate(zip((pv_e, pv_o), bcs)):
            nc.vector.tensor_tensor(
                out=aT_sb[hl * 64:hl * 64 + 64, hp, gq:gq + QW],
                in0=pv[0:64, :], in1=bc[:], op=ALU.mult,
            )

    # ---- schedule ----
    # PRE: pair-0 q/k cols [0,512) + v chunks 0..3 (everything (g0,hp0)
    # and its pv need)
    emit_qk(0, "q", 0)
    emit_qk(0, "k", 0)
    emit_v(0)
    emit_v(2)

    # filler queue: ((g, hp, i) emission position, fn), sorted.
    # Deadlines: scores(g,hp) need q(hp,g) + k(hp,0..g) emitted strictly
    # before attn_pair(hp,g); pv(g,·,i) needs v chunks 4i..4i+3 before
    # position (g,·,i) (pv lags scores by one i); outproj(t in g') needs
    # norm(g',hp1) done, i.e. position > (g',1,last).
    fillers = [
        ((0, 0, 0), lambda: emit_qk(1, "q", 0)),
        ((0, 0, 0), lambda: emit_qk(1, "k", 0)),
        ((0, 1, 0), lambda: emit_qk(0, "q", 1)),
        ((0, 1, 0), lambda: emit_qk(0, "k", 1)),
        ((1, 0, 0), lambda: emit_v(4)),
        ((1, 0, 0), lambda: emit_v(6)),
        ((1, 0, 1), lambda: emit_qk(1, "q", 1)),
        ((1, 1, 0), lambda: emit_qk(1, "k", 1)),
        ((1, 1, 0), lambda: emit_qk(0, "q", 2)),
        ((1, 1, 1), lambda: emit_qk(0, "k", 2)),
        ((1, 1, 1), lambda: emit_outproj(0, 0)),
        ((1, 1, 1), lambda: emit_outproj(0, 1)),
        ((2, 0, 0), lambda: emit_qk(1, "q", 2)),
        ((2, 0, 0), lambda: emit_qk(1, "k", 2)),
        ((2, 0, 1), lambda: emit_v(8)),
        ((2, 0, 1), lambda: emit_outproj(1, 0)),
        ((2, 0, 2), lambda: emit_v(10)),
        ((2, 0, 2), lambda: emit_outproj(1, 1)),
        ((2, 1, 0), lambda: emit_qk(0, "q", 3)),
        ((2, 1, 0), lambda: emit_outproj(2, 0)),
        ((2, 1, 1), lambda: emit_qk(0, "k", 3)),
        ((2, 1, 1), lambda: emit_outproj(2, 1)),
        ((2, 1, 2), lambda: emit_outproj(3, 0)),
        ((2, 1, 2), lambda: emit_outproj(3, 1)),
        ((3, 0, 0), lambda: emit_qk(1, "q", 3)),
        ((3, 0, 0), lambda: emit_qk(1, "k", 3)),
        ((3, 0, 1), lambda: emit_v(12)),
        ((3, 0, 1), lambda: emit_outproj(4, 0)),
        ((3, 0, 2), lambda: emit_v(14)),
        ((3, 0, 2), lambda: emit_outproj(4, 1)),
        ((3, 0, 3), lambda: emit_outproj(5, 0)),
        ((3, 0, 3), lambda: emit_outproj(5, 1)),
        ((3, 1, 0), lambda: emit_outproj(6, 0)),
        ((3, 1, 0), lambda: emit_outproj(6, 1)),
        ((3, 1, 1), lambda: emit_outproj(7, 0)),
        ((3, 1, 1), lambda: emit_outproj(7, 1)),
    ]

    for g in range(c.QG):
        for hp in range(c.MC):
            attn_pair(hp, g, fillers)
    while fillers:
        fillers.pop(0)[1]()
    for t in range(8, c.TC):
        emit_outproj(t, 0)
        emit_outproj(t, 1)


# ---------------------------------------------------------------------------
# host side
# ---------------------------------------------------------------------------

_CACHE: dict = {}


def _get_program(cfg: Cfg):
    if cfg not in _CACHE:
        _CACHE[cfg] = build_program(cfg)
    return _CACHE[cfg]


def _is_causal(mask: np.ndarray, T: int) -> bool:
    m = (np.asarray(mask).reshape(T, T) != 0)
    return np.array_equal(m, np.tril(np.ones((T, T), dtype=bool)))


def make_in_maps(cfg: Cfg, x, W_qkv, b_qkv, W_out):
    c = cfg
    bf = ml_dtypes.bfloat16
    B = x.shape[0]
    n_hg = N_CORES // B
    in_maps = []
    for core in range(N_CORES):
        b, hg = divmod(core, n_hg)
        col0 = hg * c.NHD
        xT_ = np.ascontiguousarray(x[b].T).astype(bf)
        wq_ = np.ascontiguousarray(
            W_qkv[:, 0 * c.DM + col0:0 * c.DM + col0 + c.NHD]).astype(bf)
        wk_ = np.ascontiguousarray(
            W_qkv[:, 1 * c.DM + col0:1 * c.DM + col0 + c.NHD]).astype(bf)
        wv_ = np.ascontiguousarray(
            W_qkv[:, 2 * c.DM + col0:2 * c.DM + col0 + c.NHD]).astype(bf)
        bq_ = np.ascontiguousarray(
            b_qkv[0 * c.DM + col0:0 * c.DM + col0 + c.NHD]
            .reshape(c.MC, 128).T).astype(np.float32)
        bk_ = np.ascontiguousarray(
            b_qkv[1 * c.DM + col0:1 * c.DM + col0 + c.NHD]
            .reshape(c.MC, 128).T).astype(np.float32)
        bv_ = b_qkv[2 * c.DM + col0:2 * c.DM + col0 + c.NHD].astype(np.float32)
        bvb_ = np.ascontiguousarray(np.broadcast_to(bv_, (128, c.NHD)))
        wo_ = np.ascontiguousarray(W_out[col0:col0 + c.NHD, :]).astype(bf)
        in_maps.append(dict(xT=xT_, wq=wq_, wk=wk_, wv=wv_, bq=bq_, bk=bk_,
                            bvb=bvb_, wo=wo_))
    return in_maps


def run_sharded(cfg: Cfg, x, W_qkv, b_qkv, W_out, b_out, mask=None, **kw):
    nc = _get_program(cfg)
    in_maps = make_in_maps(cfg, x, W_qkv, b_qkv, W_out)
    res = bass_utils.run_bass_kernel_spmd(
        nc, in_maps, core_ids=list(range(N_CORES)), **kw,
    )
    outs = [r["out"] for r in res.results]
    B = x.shape[0]
    n_hg = N_CORES // B
    y = np.stack([
        np.sum(outs[b * n_hg:(b + 1) * n_hg], axis=0) for b in range(B)
    ]) + b_out.astype(np.float32)
    return y.astype(np.float32), res


def _numpy_fallback(x, W_qkv, b_qkv, W_out, b_out, mask):
    B, T, C = x.shape
    H = 16
    hd = C // H
    qkv = x @ W_qkv + b_qkv
    q, k, v = np.split(qkv, 3, axis=2)
    q = q.reshape(B, T, H, hd).transpose(0, 2, 1, 3)
    k = k.reshape(B, T, H, hd).transpose(0, 2, 1, 3)
    v = v.reshape(B, T, H, hd).transpose(0, 2, 1, 3)
    s = np.einsum('bhqd,bhkd->bhqk', q, k) / math.sqrt(hd)
    s = np.where(np.asarray(mask).reshape(1, 1, T, T) == 0,
                 np.float32(-1e9), s)
    s = s - s.max(axis=-1, keepdims=True)
    e = np.exp(s)
    a = e / e.sum(axis=-1, keepdims=True)
    o = np.einsum('bhqk,bhkd->bhqd', a, v)
    o = o.transpose(0, 2, 1, 3).reshape(B, T, C)
    return (o @ W_out + b_out).astype(np.float32)


def kernel(x, W_qkv, b_qkv, W_out, b_out, mask):
    x = np.asarray(x, dtype=np.float32)
    W_qkv = np.asarray(W_qkv, dtype=np.float32)
    b_qkv = np.asarray(b_qkv, dtype=np.float32)
    W_out = np.asarray(W_out, dtype=np.float32)
    b_out = np.asarray(b_out, dtype=np.float32)
    B, T, DM = x.shape
    if not _is_causal(mask, T):
        return _numpy_fallback(x, W_qkv, b_qkv, W_out, b_out, mask)
    cfg = Cfg(T=T, DM=DM)
    y, _ = run_sharded(cfg, x, W_qkv, b_qkv, W_out, b_out, mask)
    return y


# revision 8
# speedup vs baseline: 1.0283x; 1.0194x over previous
"""Multi-head attention (B=2, T=2048, d_model=1024, H=16, hd=64) on 8 Trainium2
NeuronCores.

Sharding: 32 (batch, head) units -> 4 consecutive heads of one batch per core.
Each core: QKV projection slice, causal attention for its heads, partial
out-projection (its 256 rows of W_out). Host sums 4 partials/batch + b_out.

v2 design (vs v1):
  - scores: two concurrent half-array K=64 matmuls (head pair at partition
    bases 0/64) instead of zero-padded K=128 -> ~2x score throughput.
  - scores land in bf16 psum tiles [128, 4*512] (4 k-chunks, 2 banks) so one
    ACT exp instruction covers 4 chunks (amortizes 352-cyc ACT overhead).
  - attn@V transposed: lhsT = v[128, hd+1] stationary (cheap LDWEIGHTS),
    rhs = E streams N=512 -> psum pvT[65, 512] = aT rows + rowsum row.
    No DRAM round-trip transpose, no per-q-tile tiny matmuls.
  - diagonal narrowing: score/pv matmuls skip fully-masked column ranges;
    masked E columns are never read, so no zero fills.
  - softmax normalize in transposed space: DVE reciprocal of rowsum row,
    gpsimd partition_broadcast, DVE multiply -> aT sbuf (bf16).
  - g-outer loop with a deadline-driven filler queue (projection slices,
    out-projection t-chunks) to keep PE busy while ACT (exp) is the
    bottleneck engine.
"""

import math
from contextlib import ExitStack
from dataclasses import dataclass

import numpy as np
import ml_dtypes

import concourse.bass as bass
import concourse.tile as tile
from concourse import bacc, mybir
from concourse import bass_utils

AF = mybir.ActivationFunctionType
ALU = mybir.AluOpType
DT = mybir.dt

N_CORES = 8
NEG = -1e9


@dataclass(frozen=True)
class Cfg:
    T: int = 2048        # sequence length
    DM: int = 1024       # d_model
    HD: int = 64         # head dim
    NH: int = 4          # heads per core
    QW: int = 512        # q group width
    scores_bf16: bool = False  # bf16 psum matmul output unsupported by bass

    @property
    def NHD(self):
        return self.NH * self.HD          # qkv slice width per core

    @property
    def KC(self):
        return self.DM // 128             # contraction chunks for projections

    @property
    def MC(self):
        return self.NH // 2               # head pairs per core

    @property
    def TC(self):
        return self.T // 128              # t chunks

    @property
    def QG(self):
        return self.T // self.QW          # q groups

    @property
    def QT(self):
        return self.QW // 128             # new k chunks per q group


def build_program(cfg: Cfg):
    c = cfg
    nc = bacc.Bacc("TRN2", target_bir_lowering=False, debug=False,
                   num_devices=N_CORES)
    f32 = DT.float32
    bf16 = DT.bfloat16

    xT = nc.dram_tensor("xT", [c.DM, c.T], bf16, kind="ExternalInput").ap()
    wq = nc.dram_tensor("wq", [c.DM, c.NHD], bf16, kind="ExternalInput").ap()
    wk = nc.dram_tensor("wk", [c.DM, c.NHD], bf16, kind="ExternalInput").ap()
    wv = nc.dram_tensor("wv", [c.DM, c.NHD], bf16, kind="ExternalInput").ap()
    bq = nc.dram_tensor("bq", [128, c.MC], f32, kind="ExternalInput").ap()
    bk = nc.dram_tensor("bk", [128, c.MC], f32, kind="ExternalInput").ap()
    bvb = nc.dram_tensor("bvb", [128, c.NHD], f32, kind="ExternalInput").ap()
    wo = nc.dram_tensor("wo", [c.NHD, c.DM], bf16, kind="ExternalInput").ap()
    out = nc.dram_tensor("out", [c.T, c.DM], f32, kind="ExternalOutput").ap()

    with tile.TileContext(nc) as tc, ExitStack() as ctx:
        _body(ctx, tc, c, xT, wq, wk, wv, bq, bk, bvb, wo, out)
    nc.compile()
    return nc


def _body(ctx, tc, c: Cfg, xT, wq, wk, wv, bq, bk, bvb, wo, out):
    nc = tc.nc
    f32 = DT.float32
    bf16 = DT.bfloat16
    scale = 1.0 / math.sqrt(c.HD)
    HD1 = c.HD + 1
    QW = c.QW
    QT = c.QT

    const = ctx.enter_context(tc.tile_pool(name="const", bufs=1))
    big = ctx.enter_context(tc.tile_pool(name="big", bufs=1))
    epool = ctx.enter_context(tc.tile_pool(name="E", bufs=8))
    npool = ctx.enter_context(tc.tile_pool(name="norm", bufs=4))
    ostage = ctx.enter_context(tc.tile_pool(name="ostage", bufs=3))
    # PSUM budget (8 banks): pmm 2x2 + pfp 2x1 + ppv 2x1 = 8
    sdt = bf16 if c.scores_bf16 else f32
    SCH = 4 if c.scores_bf16 else 2          # score k-chunks per psum tile
    pmm = ctx.enter_context(tc.tile_pool(name="pmm", bufs=2, space="PSUM"))
    pfp = ctx.enter_context(tc.tile_pool(name="pfp", bufs=2, space="PSUM"))
    ppv = ctx.enter_context(tc.tile_pool(name="ppv", bufs=2, space="PSUM"))

    # ---- input DMAs ----
    bq_sb = const.tile([128, c.MC], f32, tag="bq")
    nc.sync.dma_start(out=bq_sb[:], in_=bq)
    bk_sb = const.tile([128, c.MC], f32, tag="bk")
    nc.sync.dma_start(out=bk_sb[:], in_=bk)
    bvb_sb = const.tile([128, c.NHD], f32, tag="bvb")
    nc.sync.dma_start(out=bvb_sb[:], in_=bvb)

    w_sbs = []
    for nm, w in (("wq", wq), ("wk", wk), ("wv", wv)):
        w_sb = big.tile([128, c.KC, c.NHD], bf16, tag=nm)
        nc.sync.dma_start(out=w_sb[:],
                          in_=w.rearrange("(c p) n -> p c n", p=128))
        w_sbs.append(w_sb)
    wq_sb, wk_sb, wv_sb = w_sbs

    # xT in column halves so first projections can start early
    xT_sb = big.tile([128, c.KC, c.T], bf16, tag="xT")
    xTd = xT.rearrange("(c p) t -> p c t", p=128)
    TH = max(512, c.T // 2)
    for h in range(c.T // TH):
        nc.sync.dma_start(out=xT_sb[:, :, h * TH:(h + 1) * TH],
                          in_=xTd[:, :, h * TH:(h + 1) * TH])

    wo_sb = big.tile([128, c.MC, c.DM], bf16, tag="wo")
    nc.sync.dma_start(out=wo_sb[:],
                      in_=wo.rearrange("(c p) n -> p c n", p=128))

    # causal mask block: tri[k, j] = 0 if j >= k else NEG
    tri = const.tile([128, 128], f32, tag="tri")
    nc.gpsimd.memset(tri[:], 0.0)
    nc.gpsimd.affine_select(
        out=tri[:], in_=tri[:],
        compare_op=ALU.is_ge, fill=NEG,
        base=0, channel_multiplier=-1, pattern=[[1, 128]],
    )

    # exp table-load warmup (overlaps the input DMA phase)
    warm = const.tile([1, 16], f32, tag="warm")
    nc.vector.memset(warm[:], 0.0)
    nc.scalar.activation(warm[:], warm[:], AF.Exp, scale=1.0)

    # persistent activations
    qT_sb = big.tile([128, c.MC, c.T], bf16, tag="qT")   # pair layout
    kT_sb = big.tile([128, c.MC, c.T], bf16, tag="kT")
    # v padded with 64 ones-columns: pv matmul then yields the softmax
    # rowsum in psum partitions 64..127 (64 lanes for the reciprocal,
    # instead of a 3.3us single-partition op)
    v_sb = big.tile([128, c.TC, c.NH, 128], bf16, tag="v")
    nc.vector.memset(v_sb[:, :, :, c.HD:128], 1.0)
    aT_sb = big.tile([128, c.MC, c.T], bf16, tag="aT")

    # ---- projection / out-proj emitters (pfp [128,512] one-bank tiles) ----
    def emit_qk(m, which, n):
        """q or k for pair m, columns [n*512, (n+1)*512)."""
        w_sb, b_sb = (wq_sb, bq_sb) if which == "q" else (wk_sb, bk_sb)
        ps = pfp.tile([128, 512], f32, tag="fp")
        for k in range(c.KC):
            nc.tensor.matmul(
                ps[:],
                lhsT=w_sb[:, k, m * 128:(m + 1) * 128],
                rhs=xT_sb[:, k, n * 512:(n + 1) * 512],
                start=(k == 0), stop=(k == c.KC - 1),
            )
        dst = qT_sb if which == "q" else kT_sb
        nc.vector.tensor_scalar_add(
            dst[:, m, n * 512:(n + 1) * 512], ps[:], b_sb[:, m:m + 1],
        )

    def emit_v(tg):
        """v for t-chunks [tg, tg+2)."""
        ps = pfp.tile([128, 512], f32, tag="fp")
        for d in range(2):
            t = tg + d
            for k in range(c.KC):
                nc.tensor.matmul(
                    ps[:, d * c.NHD:(d + 1) * c.NHD],
                    lhsT=xT_sb[:, k, t * 128:(t + 1) * 128],
                    rhs=wv_sb[:, k, :],
                    start=(k == 0), stop=(k == c.KC - 1),
                )
        for d in range(2):
            t = tg + d
            nc.vector.tensor_tensor(
                out=v_sb[:, t, :, 0:c.HD],
                in0=ps[:, d * c.NHD:(d + 1) * c.NHD].rearrange(
                    "p (h d) -> p h d", d=c.HD),
                in1=bvb_sb.rearrange("p (h d) -> p h d", d=c.HD),
                op=ALU.add,
            )

    def emit_outproj(t, half):
        """partial out-projection for t-chunk t, output cols half*512.
        half==1 also DMAs the completed row block."""
        ps = pfp.tile([128, 512], f32, tag="fp")
        for ci in range(c.MC):
            nc.tensor.matmul(
                ps[:],
                lhsT=aT_sb[:, ci, t * 128:(t + 1) * 128],
                rhs=wo_sb[:, ci, half * 512:(half + 1) * 512],
                start=(ci == 0), stop=(ci == c.MC - 1),
            )
        key = t % 3
        if half == 0:
            ot = ostage.tile([128, 1024], f32, tag="o")
            _ostage[key] = ot
        else:
            ot = _ostage[key]
        nc.vector.tensor_copy(ot[:, half * 512:(half + 1) * 512], ps[:])
        if half == 1:
            nc.sync.dma_start(out=out[t * 128:(t + 1) * 128, :], in_=ot[:])

    _ostage = {}

    # ---- attention ----
    def attn_pair(hp, g, fillers):
        kmax = (g + 1) * QT
        NT = -(-kmax // SCH)
        gq = g * QW
        he, ho = 2 * hp, 2 * hp + 1
        pv_e = ppv.tile([128, QW], f32, tag="pv")
        pv_o = ppv.tile([128, QW], f32, tag="pv")
        etiles = []

        def emit_scores(i):
            kc0 = i * SCH
            nch = min(SCH, kmax - kc0)
            psE = pmm.tile([128, SCH * 512], sdt, tag="mm")
            psO = pmm.tile([128, SCH * 512], sdt, tag="mm")
            for j in range(nch):
                kc = kc0 + j
                off = max(0, (kc - g * QT) * 128)   # fully-masked prefix
                for base, ps in ((0, psE), (64, psO)):
                    nc.tensor.matmul(
                        ps[:, j * 512 + off:(j + 1) * 512],
                        lhsT=kT_sb[base:base + 64, hp,
                                   kc * 128:(kc + 1) * 128],
                        rhs=qT_sb[base:base + 64, hp, gq + off:gq + QW],
                        start=True, stop=True,
                    )
            for j in range(nch):
                kc = kc0 + j
                off = (kc - g * QT) * 128
                if off >= 0:                        # diagonal chunk
                    for ps in (psE, psO):
                        nc.vector.tensor_tensor(
                            out=ps[:, j * 512 + off:j * 512 + off + 128],
                            in0=ps[:, j * 512 + off:j * 512 + off + 128],
                            in1=tri[:], op=ALU.add,
                        )
            # exp; split per-chunk when the fully-masked prefixes are wide
            # enough to outweigh one extra ACT instruction (~352 cyc)
            offs = [max(0, (kc0 + j - g * QT) * 128) for j in range(nch)]
            ee = epool.tile([128, SCH * 512], bf16, tag="E")
            eo = epool.tile([128, SCH * 512], bf16, tag="E")
            for et, ps in ((ee, psE), (eo, psO)):
                if sum(offs) > 512:
                    for j in range(nch):
                        nc.scalar.activation(
                            et[:, j * 512 + offs[j]:(j + 1) * 512],
                            ps[:, j * 512 + offs[j]:(j + 1) * 512],
                            AF.Exp, scale=scale)
                else:
                    nc.scalar.activation(et[:, 0:nch * 512],
                                         ps[:, 0:nch * 512],
                                         AF.Exp, scale=scale)
            etiles.append((ee, eo, kc0, nch))

        def emit_pv(i):
            ee, eo, kc0, nch = etiles[i]
            for et, h, pv in ((ee, he, pv_e), (eo, ho, pv_o)):
                for j in range(nch):
                    kc = kc0 + j
                    off = max(0, (kc - g * QT) * 128)
                    nc.tensor.matmul(
                        pv[:, off:QW],
                        lhsT=v_sb[:, kc, h, :],
                        rhs=et[:, j * 512 + off:(j + 1) * 512],
                        start=(kc == 0), stop=(kc == kmax - 1),
                    )

        for i in range(NT):
            emit_scores(i)
            while fillers and fillers[0][0] <= (g, hp, i):
                fillers.pop(0)[1]()
            if i >= 1:
                emit_pv(i - 1)
        emit_pv(NT - 1)

        # normalize: aT[d, q] = pvT[d, q] * (1 / rowsum[q]); rowsum is
        # replicated in psum partitions 64..127 by the ones block of v
        for hl, pv in enumerate((pv_e, pv_o)):
            rc = npool.tile([64, QW], f32, tag="rc")
            nc.vector.reciprocal(rc[:], pv[64:128, :])
            nc.vector.tensor_tensor(
                out=aT_sb[hl * 64:hl * 64 + 64, hp, gq:gq + QW],
                in0=pv[0:64, :], in1=rc[:], op=ALU.mult,
            )

    # ---- schedule ----
    # PRE: pair-0 q/k cols [0,512) + v chunks 0..3 (everything (g0,hp0)
    # and its pv need)
    emit_qk(0, "q", 0)
    emit_qk(0, "k", 0)
    emit_v(0)
    emit_v(2)

    # filler queue: ((g, hp, i) emission position, fn), sorted.
    # Deadlines: scores(g,hp) need q(hp,g) + k(hp,0..g) emitted strictly
    # before attn_pair(hp,g); pv(g,·,i) needs v chunks 4i..4i+3 before
    # position (g,·,i) (pv lags scores by one i); outproj(t in g') needs
    # norm(g',hp1) done, i.e. position > (g',1,last).
    fillers = [
        ((0, 0, 0), lambda: emit_qk(1, "q", 0)),
        ((0, 0, 0), lambda: emit_qk(1, "k", 0)),
        ((0, 1, 0), lambda: emit_qk(0, "q", 1)),
        ((0, 1, 0), lambda: emit_qk(0, "k", 1)),
        ((1, 0, 0), lambda: emit_v(4)),
        ((1, 0, 0), lambda: emit_v(6)),
        ((1, 0, 1), lambda: emit_qk(1, "q", 1)),
        ((1, 1, 0), lambda: emit_qk(1, "k", 1)),
        ((1, 1, 0), lambda: emit_qk(0, "q", 2)),
        ((1, 1, 1), lambda: emit_qk(0, "k", 2)),
        ((1, 1, 1), lambda: emit_outproj(0, 0)),
        ((1, 1, 1), lambda: emit_outproj(0, 1)),
        ((2, 0, 0), lambda: emit_qk(1, "q", 2)),
        ((2, 0, 0), lambda: emit_qk(1, "k", 2)),
        ((2, 0, 1), lambda: emit_v(8)),
        ((2, 0, 1), lambda: emit_outproj(1, 0)),
        ((2, 0, 2), lambda: emit_v(10)),
        ((2, 0, 2), lambda: emit_outproj(1, 1)),
        ((2, 1, 0), lambda: emit_qk(0, "q", 3)),
        ((2, 1, 0), lambda: emit_outproj(2, 0)),
        ((2, 1, 1), lambda: emit_qk(0, "k", 3)),
        ((2, 1, 1), lambda: emit_outproj(2, 1)),
        ((2, 1, 2), lambda: emit_outproj(3, 0)),
        ((2, 1, 2), lambda: emit_outproj(3, 1)),
        ((3, 0, 0), lambda: emit_qk(1, "q", 3)),
        ((3, 0, 0), lambda: emit_qk(1, "k", 3)),
        ((3, 0, 1), lambda: emit_v(12)),
        ((3, 0, 1), lambda: emit_outproj(4, 0)),
        ((3, 0, 2), lambda: emit_v(14)),
        ((3, 0, 2), lambda: emit_outproj(4, 1)),
        ((3, 0, 3), lambda: emit_outproj(5, 0)),
        ((3, 0, 3), lambda: emit_outproj(5, 1)),
        ((3, 1, 0), lambda: emit_outproj(6, 0)),
        ((3, 1, 0), lambda: emit_outproj(6, 1)),
        ((3, 1, 1), lambda: emit_outproj(7, 0)),
        ((3, 1, 1), lambda: emit_outproj(7, 1)),
    ]

    for g in range(c.QG):
        for hp in range(c.MC):
            attn_pair(hp, g, fillers)
    while fillers:
        fillers.pop(0)[1]()
    for t in range(8, c.TC):
        emit_outproj(t, 0)
        emit_outproj(t, 1)


# ---------------------------------------------------------------------------
# host side
# ---------------------------------------------------------------------------

_CACHE: dict = {}


def _get_program(cfg: Cfg):
    if cfg not in _CACHE:
        _CACHE[cfg] = build_program(cfg)
    return _CACHE[cfg]


def _is_causal(mask: np.ndarray, T: int) -> bool:
    m = (np.asarray(mask).reshape(T, T) != 0)
    return np.array_equal(m, np.tril(np.ones((T, T), dtype=bool)))


def make_in_maps(cfg: Cfg, x, W_qkv, b_qkv, W_out):
    c = cfg
    bf = ml_dtypes.bfloat16
    B = x.shape[0]
    n_hg = N_CORES // B
    in_maps = []
    for core in range(N_CORES):
        b, hg = divmod(core, n_hg)
        col0 = hg * c.NHD
        xT_ = np.ascontiguousarray(x[b].T).astype(bf)
        wq_ = np.ascontiguousarray(
            W_qkv[:, 0 * c.DM + col0:0 * c.DM + col0 + c.NHD]).astype(bf)
        wk_ = np.ascontiguousarray(
            W_qkv[:, 1 * c.DM + col0:1 * c.DM + col0 + c.NHD]).astype(bf)
        wv_ = np.ascontiguousarray(
            W_qkv[:, 2 * c.DM + col0:2 * c.DM + col0 + c.NHD]).astype(bf)
        bq_ = np.ascontiguousarray(
            b_qkv[0 * c.DM + col0:0 * c.DM + col0 + c.NHD]
            .reshape(c.MC, 128).T).astype(np.float32)
        bk_ = np.ascontiguousarray(
            b_qkv[1 * c.DM + col0:1 * c.DM + col0 + c.NHD]
            .reshape(c.MC, 128).T).astype(np.float32)
        bv_ = b_qkv[2 * c.DM + col0:2 * c.DM + col0 + c.NHD].astype(np.float32)
        bvb_ = np.ascontiguousarray(np.broadcast_to(bv_, (128, c.NHD)))
        wo_ = np.ascontiguousarray(W_out[col0:col0 + c.NHD, :]).astype(bf)
        in_maps.append(dict(xT=xT_, wq=wq_, wk=wk_, wv=wv_, bq=bq_, bk=bk_,
                            bvb=bvb_, wo=wo_))
    return in_maps


def run_sharded(cfg: Cfg, x, W_qkv, b_qkv, W_out, b_out, mask=None, **kw):
    nc = _get_program(cfg)
    in_maps = make_in_maps(cfg, x, W_qkv, b_qkv, W_out)
    res = bass_utils.run_bass_kernel_spmd(
        nc, in_maps, core_ids=list(range(N_CORES)), **kw,
    )
    outs = [r["out"] for r in res.results]
    B = x.shape[0]
    n_hg = N_CORES // B
    y = np.stack([
        np.sum(outs[b * n_hg:(b + 1) * n_hg], axis=0) for b in range(B)
    ]) + b_out.astype(np.float32)
    return y.astype(np.float32), res


def _numpy_fallback(x, W_qkv, b_qkv, W_out, b_out, mask):
    B, T, C = x.shape
    H = 16
    hd = C // H
    qkv = x @ W_qkv + b_qkv
    q, k, v = np.split(qkv, 3, axis=2)
    q = q.reshape(B, T, H, hd).transpose(0, 2, 1, 3)
    k = k.reshape(B, T, H, hd).transpose(0, 2, 1, 3)
    v = v.reshape(B, T, H, hd).transpose(0, 2, 1, 3)
    s = np.einsum('bhqd,bhkd->bhqk', q, k) / math.sqrt(hd)
    s = np.where(np.asarray(mask).reshape(1, 1, T, T) == 0,
                 np.float32(-1e9), s)
    s = s - s.max(axis=-1, keepdims=True)
    e = np.exp(s)
    a = e / e.sum(axis=-1, keepdims=True)
    o = np.einsum('bhqk,bhkd->bhqd', a, v)
    o = o.transpose(0, 2, 1, 3).reshape(B, T, C)
    return (o @ W_out + b_out).astype(np.float32)


def kernel(x, W_qkv, b_qkv, W_out, b_out, mask):
    x = np.asarray(x, dtype=np.float32)
    W_qkv = np.asarray(W_qkv, dtype=np.float32)
    b_qkv = np.asarray(b_qkv, dtype=np.float32)
    W_out = np.asarray(W_out, dtype=np.float32)
    b_out = np.asarray(b_out, dtype=np.float32)
    B, T, DM = x.shape
    if not _is_causal(mask, T):
        return _numpy_fallback(x, W_qkv, b_qkv, W_out, b_out, mask)
    cfg = Cfg(T=T, DM=DM)
    y, _ = run_sharded(cfg, x, W_qkv, b_qkv, W_out, b_out, mask)
    return y


# revision 15
# speedup vs baseline: 1.3904x; 1.3520x over previous
"""Multi-head attention (B=2, T=2048, d_model=1024, H=16, hd=64) on 8 Trainium2
NeuronCores.

Sharding: 32 (batch, head) units -> 4 consecutive heads of one batch per core.
Each core: QKV projection slice, causal attention for its heads, partial
out-projection (its 256 rows of W_out). Host sums 4 partials/batch + b_out.

v2 design (vs v1):
  - scores: two concurrent half-array K=64 matmuls (head pair at partition
    bases 0/64) instead of zero-padded K=128 -> ~2x score throughput.
  - scores land in bf16 psum tiles [128, 4*512] (4 k-chunks, 2 banks) so one
    ACT exp instruction covers 4 chunks (amortizes 352-cyc ACT overhead).
  - attn@V transposed: lhsT = v[128, hd+1] stationary (cheap LDWEIGHTS),
    rhs = E streams N=512 -> psum pvT[65, 512] = aT rows + rowsum row.
    No DRAM round-trip transpose, no per-q-tile tiny matmuls.
  - diagonal narrowing: score/pv matmuls skip fully-masked column ranges;
    masked E columns are never read, so no zero fills.
  - softmax normalize in transposed space: DVE reciprocal of rowsum row,
    gpsimd partition_broadcast, DVE multiply -> aT sbuf (bf16).
  - g-outer loop with a deadline-driven filler queue (projection slices,
    out-projection t-chunks) to keep PE busy while ACT (exp) is the
    bottleneck engine.
"""

import math
from contextlib import ExitStack
from dataclasses import dataclass

import numpy as np
import ml_dtypes

import concourse.bass as bass
import concourse.tile as tile
from concourse import bacc, mybir
from concourse import bass_utils

AF = mybir.ActivationFunctionType
ALU = mybir.AluOpType
DT = mybir.dt

N_CORES = 8
NEG = -1e9


@dataclass(frozen=True)
class Cfg:
    T: int = 2048        # sequence length
    DM: int = 1024       # d_model
    HD: int = 64         # head dim
    NH: int = 4          # heads per core
    QW: int = 512        # q group width
    scores_bf16: bool = False  # bf16 psum matmul output unsupported by bass

    @property
    def NHD(self):
        return self.NH * self.HD          # qkv slice width per core

    @property
    def KC(self):
        return self.DM // 128             # contraction chunks for projections

    @property
    def MC(self):
        return self.NH // 2               # head pairs per core

    @property
    def TC(self):
        return self.T // 128              # t chunks

    @property
    def QG(self):
        return self.T // self.QW          # q groups

    @property
    def QT(self):
        return self.QW // 128             # new k chunks per q group


def build_program(cfg: Cfg):
    c = cfg
    nc = bacc.Bacc("TRN2", target_bir_lowering=False, debug=False,
                   num_devices=N_CORES)
    f32 = DT.float32
    bf16 = DT.bfloat16

    xT = nc.dram_tensor("xT", [c.DM, c.T], bf16, kind="ExternalInput").ap()
    wq = nc.dram_tensor("wq", [c.DM, c.NHD], bf16, kind="ExternalInput").ap()
    wk = nc.dram_tensor("wk", [c.DM, c.NHD], bf16, kind="ExternalInput").ap()
    wv = nc.dram_tensor("wv", [c.DM, c.NHD], bf16, kind="ExternalInput").ap()
    bq = nc.dram_tensor("bq", [128, c.MC], f32, kind="ExternalInput").ap()
    bk = nc.dram_tensor("bk", [128, c.MC], f32, kind="ExternalInput").ap()
    bvb = nc.dram_tensor("bvb", [128, c.NHD], f32, kind="ExternalInput").ap()
    wo = nc.dram_tensor("wo", [c.NHD, c.DM], bf16, kind="ExternalInput").ap()
    out = nc.dram_tensor("out", [c.T, c.DM], f32, kind="ExternalOutput").ap()

    with tile.TileContext(nc) as tc, ExitStack() as ctx:
        _body(ctx, tc, c, xT, wq, wk, wv, bq, bk, bvb, wo, out)
    nc.compile()
    return nc


def _body(ctx, tc, c: Cfg, xT, wq, wk, wv, bq, bk, bvb, wo, out):
    nc = tc.nc
    f32 = DT.float32
    bf16 = DT.bfloat16
    scale = 1.0 / math.sqrt(c.HD)
    HD1 = c.HD + 1
    QW = c.QW
    QT = c.QT

    const = ctx.enter_context(tc.tile_pool(name="const", bufs=1))
    big = ctx.enter_context(tc.tile_pool(name="big", bufs=1))
    epool = ctx.enter_context(tc.tile_pool(name="E", bufs=8))
    npool = ctx.enter_context(tc.tile_pool(name="norm", bufs=4))
    ostage = ctx.enter_context(tc.tile_pool(name="ostage", bufs=3))
    # PSUM budget (8 banks): pmm 2x2 + pfp 2x1 + ppv 2x1 = 8
    sdt = bf16 if c.scores_bf16 else f32
    SCH = 4 if c.scores_bf16 else 2          # score k-chunks per psum tile
    pmm = ctx.enter_context(tc.tile_pool(name="pmm", bufs=2, space="PSUM"))
    pfp = ctx.enter_context(tc.tile_pool(name="pfp", bufs=2, space="PSUM"))
    ppv = ctx.enter_context(tc.tile_pool(name="ppv", bufs=2, space="PSUM"))

    # ---- input DMAs, ordered by first use: wq/wk -> xT[0:512] -> rest ----
    wq_sb = big.tile([128, c.KC, c.NHD], bf16, tag="wq")
    nc.sync.dma_start(out=wq_sb[:],
                      in_=wq.rearrange("(c p) n -> p c n", p=128))
    wk_sb = big.tile([128, c.KC, c.NHD], bf16, tag="wk")
    nc.sync.dma_start(out=wk_sb[:],
                      in_=wk.rearrange("(c p) n -> p c n", p=128))

    xT_sb = big.tile([128, c.KC, c.T], bf16, tag="xT")
    xTd = xT.rearrange("(c p) t -> p c t", p=128)
    nc.sync.dma_start(out=xT_sb[:, :, 0:512], in_=xTd[:, :, 0:512])

    bq_sb = const.tile([128, c.MC], f32, tag="bq")
    nc.sync.dma_start(out=bq_sb[:], in_=bq)
    bk_sb = const.tile([128, c.MC], f32, tag="bk")
    nc.sync.dma_start(out=bk_sb[:], in_=bk)

    wv_sb = big.tile([128, c.KC, c.NHD], bf16, tag="wv")
    nc.sync.dma_start(out=wv_sb[:],
                      in_=wv.rearrange("(c p) n -> p c n", p=128))
    bvb_sb = const.tile([128, c.NHD], f32, tag="bvb")
    nc.sync.dma_start(out=bvb_sb[:], in_=bvb)

    for sl in range(1, 4):
        nc.sync.dma_start(out=xT_sb[:, :, sl * 512:(sl + 1) * 512],
                          in_=xTd[:, :, sl * 512:(sl + 1) * 512])

    wo_sb = big.tile([128, c.MC, c.DM], bf16, tag="wo")
    nc.sync.dma_start(out=wo_sb[:],
                      in_=wo.rearrange("(c p) n -> p c n", p=128))

    # mask constants for PE-side mask accumulation (bf16):
    #   trineg[k, j] = 0 if j >= k else NEG;  ident = I
    # diagonal score blocks get  ps += ident.T @ trineg = trineg  as a
    # second accumulating matmul instead of a DVE add.
    trineg = const.tile([128, 128], bf16, tag="trineg")
    nc.gpsimd.memset(trineg[:], 0.0)
    nc.gpsimd.affine_select(
        out=trineg[:], in_=trineg[:],
        compare_op=ALU.is_ge, fill=NEG,
        base=0, channel_multiplier=-1, pattern=[[1, 128]],
    )
    ident = const.tile([128, 128], bf16, tag="ident")
    nc.gpsimd.memset(ident[:], 1.0)
    nc.gpsimd.affine_select(
        out=ident[:], in_=ident[:],
        compare_op=ALU.is_equal, fill=0.0,
        base=0, channel_multiplier=-1, pattern=[[1, 128]],
    )

    # exp table-load warmup (overlaps the input DMA phase)
    warm = const.tile([1, 16], f32, tag="warm")
    nc.vector.memset(warm[:], 0.0)
    nc.scalar.activation(warm[:], warm[:], AF.Exp, scale=1.0)

    # PE warmup: junk matmuls during the input-DMA wait so the HAM clock
    # gate is already at 8/8 when the first projection runs (~3.4us ramp)
    wsrc = const.tile([128, 512], bf16, tag="wsrc")
    nc.vector.memset(wsrc[:], 0.5)
    wps = ppv.tile([128, 512], f32, tag="pv")
    for _ in range(12):
        nc.tensor.matmul(wps[:], lhsT=wsrc[:, 0:128], rhs=wsrc[:],
                         start=True, stop=True)

    # persistent activations
    qT_sb = big.tile([128, c.MC, c.T], bf16, tag="qT")   # pair layout
    kT_sb = big.tile([128, c.MC, c.T], bf16, tag="kT")
    # v padded with 64 ones-columns: pv matmul then yields the softmax
    # rowsum in psum partitions 64..127 (64 lanes for the reciprocal,
    # instead of a 3.3us single-partition op)
    v_sb = big.tile([128, c.TC, c.NH, 128], bf16, tag="v")
    nc.vector.memset(v_sb[:, :, :, c.HD:128], 1.0)
    aT_sb = big.tile([128, c.MC, c.T], bf16, tag="aT")

    # ---- projection / out-proj emitters (pfp [128,512] one-bank tiles) ----
    def emit_qk(m, which, n):
        """q or k for pair m, columns [n*512, (n+1)*512)."""
        w_sb, b_sb = (wq_sb, bq_sb) if which == "q" else (wk_sb, bk_sb)
        ps = pfp.tile([128, 512], f32, tag="fp")
        for k in range(c.KC):
            nc.tensor.matmul(
                ps[:],
                lhsT=w_sb[:, k, m * 128:(m + 1) * 128],
                rhs=xT_sb[:, k, n * 512:(n + 1) * 512],
                start=(k == 0), stop=(k == c.KC - 1),
            )
        dst = qT_sb if which == "q" else kT_sb
        nc.vector.tensor_scalar_add(
            dst[:, m, n * 512:(n + 1) * 512], ps[:], b_sb[:, m:m + 1],
        )

    def emit_v(tg):
        """v for t-chunks [tg, tg+2)."""
        ps = pfp.tile([128, 512], f32, tag="fp")
        for d in range(2):
            t = tg + d
            for k in range(c.KC):
                nc.tensor.matmul(
                    ps[:, d * c.NHD:(d + 1) * c.NHD],
                    lhsT=xT_sb[:, k, t * 128:(t + 1) * 128],
                    rhs=wv_sb[:, k, :],
                    start=(k == 0), stop=(k == c.KC - 1),
                )
        for d in range(2):
            t = tg + d
            nc.vector.tensor_tensor(
                out=v_sb[:, t, :, 0:c.HD],
                in0=ps[:, d * c.NHD:(d + 1) * c.NHD].rearrange(
                    "p (h d) -> p h d", d=c.HD),
                in1=bvb_sb.rearrange("p (h d) -> p h d", d=c.HD),
                op=ALU.add,
            )

    def emit_outproj(t, half):
        """partial out-projection for t-chunk t, output cols half*512.
        half==1 also DMAs the completed row block."""
        ps = pfp.tile([128, 512], f32, tag="fp")
        for ci in range(c.MC):
            nc.tensor.matmul(
                ps[:],
                lhsT=aT_sb[:, ci, t * 128:(t + 1) * 128],
                rhs=wo_sb[:, ci, half * 512:(half + 1) * 512],
                start=(ci == 0), stop=(ci == c.MC - 1),
            )
        key = t % 3
        if half == 0:
            ot = ostage.tile([128, 1024], f32, tag="o")
            _ostage[key] = ot
        else:
            ot = _ostage[key]
        nc.vector.tensor_copy(ot[:, half * 512:(half + 1) * 512], ps[:])
        if half == 1:
            nc.sync.dma_start(out=out[t * 128:(t + 1) * 128, :], in_=ot[:])

    _ostage = {}

    # ---- attention ----
    def attn_pair(hp, g, fillers):
        kmax = (g + 1) * QT
        NT = -(-kmax // SCH)
        gq = g * QW
        he, ho = 2 * hp, 2 * hp + 1
        pv_e = ppv.tile([128, QW], f32, tag="pv")
        pv_o = ppv.tile([128, QW], f32, tag="pv")
        etiles = []

        def emit_scores(i):
            kc0 = i * SCH
            nch = min(SCH, kmax - kc0)
            psE = pmm.tile([128, SCH * 512], sdt, tag="mm")
            psO = pmm.tile([128, SCH * 512], sdt, tag="mm")
            for j in range(nch):
                kc = kc0 + j
                diag = (kc - g * QT) * 128 >= 0     # diagonal chunk?
                off = max(0, (kc - g * QT) * 128)   # fully-masked prefix
                for base, ps in ((0, psE), (64, psO)):
                    nc.tensor.matmul(
                        ps[:, j * 512 + off:(j + 1) * 512],
                        lhsT=kT_sb[base:base + 64, hp,
                                   kc * 128:(kc + 1) * 128],
                        rhs=qT_sb[base:base + 64, hp, gq + off:gq + QW],
                        start=True, stop=not diag,
                    )
                if diag:
                    # ps[:, off:off+128] += I.T @ trineg  (causal mask)
                    for ps in (psE, psO):
                        nc.tensor.matmul(
                            ps[:, j * 512 + off:j * 512 + off + 128],
                            lhsT=ident[:], rhs=trineg[:],
                            start=False, stop=True,
                        )
            # exp; split per-chunk when the fully-masked prefixes are wide
            # enough to outweigh one extra ACT instruction (~352 cyc)
            offs = [max(0, (kc0 + j - g * QT) * 128) for j in range(nch)]
            ee = epool.tile([128, SCH * 512], bf16, tag="E")
            eo = epool.tile([128, SCH * 512], bf16, tag="E")
            for et, ps in ((ee, psE), (eo, psO)):
                if sum(offs) > 512:
                    for j in range(nch):
                        nc.scalar.activation(
                            et[:, j * 512 + offs[j]:(j + 1) * 512],
                            ps[:, j * 512 + offs[j]:(j + 1) * 512],
                            AF.Exp, scale=scale)
                else:
                    nc.scalar.activation(et[:, 0:nch * 512],
                                         ps[:, 0:nch * 512],
                                         AF.Exp, scale=scale)
            etiles.append((ee, eo, kc0, nch))

        def emit_pv(i):
            ee, eo, kc0, nch = etiles[i]
            for et, h, pv in ((ee, he, pv_e), (eo, ho, pv_o)):
                for j in range(nch):
                    kc = kc0 + j
                    off = max(0, (kc - g * QT) * 128)
                    nc.tensor.matmul(
                        pv[:, off:QW],
                        lhsT=v_sb[:, kc, h, :],
                        rhs=et[:, j * 512 + off:(j + 1) * 512],
                        start=(kc == 0), stop=(kc == kmax - 1),
                    )

        for i in range(NT):
            emit_scores(i)
            while fillers and fillers[0][0] <= (g, hp, i):
                fillers.pop(0)[1]()
            if i >= 1:
                emit_pv(i - 1)
        emit_pv(NT - 1)

        # normalize: aT[d, q] = pvT[d, q] * (1 / rowsum[q]); rowsum is
        # replicated in psum partitions 64..127 by the ones block of v
        for hl, pv in enumerate((pv_e, pv_o)):
            # reciprocal_approx_fast is a bitwise custom DVE op — stage the
            # rowsums through SBUF (PSUM input returns garbage)
            sums = npool.tile([64, QW], f32, tag="sums")
            nc.vector.tensor_copy(sums[:], pv[64:128, :])
            rc = npool.tile([64, QW], f32, tag="rc")
            nc.vector.reciprocal_approx_fast(rc[:], sums[:])
            nc.vector.tensor_tensor(
                out=aT_sb[hl * 64:hl * 64 + 64, hp, gq:gq + QW],
                in0=pv[0:64, :], in1=rc[:], op=ALU.mult,
            )

    # ---- schedule ----
    # PRE: pair-0 q/k cols [0,512) + v chunks 0..1 ((g0,hp0) needs)
    emit_qk(0, "q", 0)
    emit_qk(0, "k", 0)
    emit_v(0)

    # filler queue: ((g, hp, i) emission position, fn), sorted.
    # Deadlines: scores(g,hp) need q(hp,g-col) + k(hp, cols <= g) emitted
    # strictly before attn_pair(hp,g); pv(g,·,i) needs v chunks 2i..2i+1
    # before position (g,·,i+1) (pv lags scores by one i); outproj(t in
    # g') needs norm(g',hp1), i.e. position >= (g'+1, 0, 0).
    fillers = [
        ((0, 0, 0), lambda: emit_v(2)),
        ((0, 0, 0), lambda: emit_qk(1, "q", 0)),
        ((0, 0, 1), lambda: emit_qk(1, "k", 0)),
        ((0, 1, 0), lambda: emit_qk(0, "q", 1)),
        ((0, 1, 0), lambda: emit_qk(0, "k", 1)),
        ((0, 1, 1), lambda: emit_v(4)),
        ((1, 0, 0), lambda: emit_v(6)),
        ((1, 0, 0), lambda: emit_qk(1, "q", 1)),
        ((1, 0, 1), lambda: emit_qk(1, "k", 1)),
        ((1, 1, 0), lambda: emit_qk(0, "q", 2)),
        ((1, 1, 1), lambda: emit_qk(0, "k", 2)),
        ((1, 1, 2), lambda: emit_v(8)),
        ((1, 1, 3), lambda: emit_v(10)),
        ((2, 0, 0), lambda: emit_qk(1, "q", 2)),
        ((2, 0, 0), lambda: emit_outproj(0, 0)),
        ((2, 0, 1), lambda: emit_qk(1, "k", 2)),
        ((2, 0, 1), lambda: emit_outproj(0, 1)),
        ((2, 0, 2), lambda: emit_outproj(1, 0)),
        ((2, 0, 3), lambda: emit_outproj(1, 1)),
        ((2, 0, 4), lambda: emit_outproj(2, 0)),
        ((2, 0, 5), lambda: emit_outproj(2, 1)),
        ((2, 1, 0), lambda: emit_qk(0, "q", 3)),
        ((2, 1, 0), lambda: emit_outproj(3, 0)),
        ((2, 1, 1), lambda: emit_qk(0, "k", 3)),
        ((2, 1, 1), lambda: emit_outproj(3, 1)),
        ((2, 1, 2), lambda: emit_outproj(4, 0)),
        ((2, 1, 3), lambda: emit_outproj(4, 1)),
        ((2, 1, 4), lambda: emit_outproj(5, 0)),
        ((2, 1, 5), lambda: emit_outproj(5, 1)),
        ((3, 0, 0), lambda: emit_qk(1, "q", 3)),
        ((3, 0, 0), lambda: emit_outproj(6, 0)),
        ((3, 0, 1), lambda: emit_qk(1, "k", 3)),
        ((3, 0, 1), lambda: emit_outproj(6, 1)),
        ((3, 0, 2), lambda: emit_v(12)),
        ((3, 0, 2), lambda: emit_outproj(7, 0)),
        ((3, 0, 3), lambda: emit_v(14)),
        ((3, 0, 3), lambda: emit_outproj(7, 1)),
        ((3, 0, 4), lambda: emit_outproj(8, 0)),
        ((3, 0, 5), lambda: emit_outproj(8, 1)),
        ((3, 0, 6), lambda: emit_outproj(9, 0)),
        ((3, 0, 7), lambda: emit_outproj(9, 1)),
        ((3, 1, 0), lambda: emit_outproj(10, 0)),
        ((3, 1, 1), lambda: emit_outproj(10, 1)),
        ((3, 1, 2), lambda: emit_outproj(11, 0)),
        ((3, 1, 3), lambda: emit_outproj(11, 1)),
    ]

    for g in range(c.QG):
        for hp in range(c.MC):
            attn_pair(hp, g, fillers)
    while fillers:
        fillers.pop(0)[1]()
    for t in range(12, c.TC):
        emit_outproj(t, 0)
        emit_outproj(t, 1)


# ---------------------------------------------------------------------------
# host side
# ---------------------------------------------------------------------------

_CACHE: dict = {}


def _get_program(cfg: Cfg):
    if cfg not in _CACHE:
        _CACHE[cfg] = build_program(cfg)
    return _CACHE[cfg]


def _is_causal(mask: np.ndarray, T: int) -> bool:
    m = (np.asarray(mask).reshape(T, T) != 0)
    return np.array_equal(m, np.tril(np.ones((T, T), dtype=bool)))


def make_in_maps(cfg: Cfg, x, W_qkv, b_qkv, W_out):
    c = cfg
    bf = ml_dtypes.bfloat16
    B = x.shape[0]
    n_hg = N_CORES // B
    in_maps = []
    for core in range(N_CORES):
        b, hg = divmod(core, n_hg)
        col0 = hg * c.NHD
        xT_ = np.ascontiguousarray(x[b].T).astype(bf)
        wq_ = np.ascontiguousarray(
            W_qkv[:, 0 * c.DM + col0:0 * c.DM + col0 + c.NHD]).astype(bf)
        wk_ = np.ascontiguousarray(
            W_qkv[:, 1 * c.DM + col0:1 * c.DM + col0 + c.NHD]).astype(bf)
        wv_ = np.ascontiguousarray(
            W_qkv[:, 2 * c.DM + col0:2 * c.DM + col0 + c.NHD]).astype(bf)
        bq_ = np.ascontiguousarray(
            b_qkv[0 * c.DM + col0:0 * c.DM + col0 + c.NHD]
            .reshape(c.MC, 128).T).astype(np.float32)
        bk_ = np.ascontiguousarray(
            b_qkv[1 * c.DM + col0:1 * c.DM + col0 + c.NHD]
            .reshape(c.MC, 128).T).astype(np.float32)
        bv_ = b_qkv[2 * c.DM + col0:2 * c.DM + col0 + c.NHD].astype(np.float32)
        bvb_ = np.ascontiguousarray(np.broadcast_to(bv_, (128, c.NHD)))
        wo_ = np.ascontiguousarray(W_out[col0:col0 + c.NHD, :]).astype(bf)
        in_maps.append(dict(xT=xT_, wq=wq_, wk=wk_, wv=wv_, bq=bq_, bk=bk_,
                            bvb=bvb_, wo=wo_))
    return in_maps


def run_sharded(cfg: Cfg, x, W_qkv, b_qkv, W_out, b_out, mask=None, **kw):
    nc = _get_program(cfg)
    in_maps = make_in_maps(cfg, x, W_qkv, b_qkv, W_out)
    res = bass_utils.run_bass_kernel_spmd(
        nc, in_maps, core_ids=list(range(N_CORES)), **kw,
    )
    outs = [r["out"] for r in res.results]
    B = x.shape[0]
    n_hg = N_CORES // B
    y = np.stack([
        np.sum(outs[b * n_hg:(b + 1) * n_hg], axis=0) for b in range(B)
    ]) + b_out.astype(np.float32)
    return y.astype(np.float32), res


def _numpy_fallback(x, W_qkv, b_qkv, W_out, b_out, mask):
    B, T, C = x.shape
    H = 16
    hd = C // H
    qkv = x @ W_qkv + b_qkv
    q, k, v = np.split(qkv, 3, axis=2)
    q = q.reshape(B, T, H, hd).transpose(0, 2, 1, 3)
    k = k.reshape(B, T, H, hd).transpose(0, 2, 1, 3)
    v = v.reshape(B, T, H, hd).transpose(0, 2, 1, 3)
    s = np.einsum('bhqd,bhkd->bhqk', q, k) / math.sqrt(hd)
    s = np.where(np.asarray(mask).reshape(1, 1, T, T) == 0,
                 np.float32(-1e9), s)
    s = s - s.max(axis=-1, keepdims=True)
    e = np.exp(s)
    a = e / e.sum(axis=-1, keepdims=True)
    o = np.einsum('bhqk,bhkd->bhqd', a, v)
    o = o.transpose(0, 2, 1, 3).reshape(B, T, C)
    return (o @ W_out + b_out).astype(np.float32)


def kernel(x, W_qkv, b_qkv, W_out, b_out, mask):
    x = np.asarray(x, dtype=np.float32)
    W_qkv = np.asarray(W_qkv, dtype=np.float32)
    b_qkv = np.asarray(b_qkv, dtype=np.float32)
    W_out = np.asarray(W_out, dtype=np.float32)
    b_out = np.asarray(b_out, dtype=np.float32)
    B, T, DM = x.shape
    if not _is_causal(mask, T):
        return _numpy_fallback(x, W_qkv, b_qkv, W_out, b_out, mask)
    cfg = Cfg(T=T, DM=DM)
    y, _ = run_sharded(cfg, x, W_qkv, b_qkv, W_out, b_out, mask)
    return y
